# revision 30
# baseline (speedup 1.0000x reference)
"""ButterflyMlp Trainium2 kernel (banded-permutation version).

Reference computation (B=65536):
    h1 = relu(x @ (W1*m1).T + b1)          # [B, 784]
    h2 = relu(h1 @ (W2*m2).T + b2)         # [B, 128]
    logits = h2 @ (W3*m3).T + b3           # [B, 10]
    out = log_softmax(logits, axis=1)

Strategy: pure data parallel over 8 NeuronCores (batch sharded 8192/core,
masked weights replicated), fp8e4m3 with fp32 PSUM accumulation.

Every nonzero offset d = j - o of the butterfly mask m1 satisfies
(d mod 156) in [-10, 10]: the band |d| <= 10 trivially, and the stripes
have d mod 156 in {0,1,2}.  Sorting both the 784 input features and the
784 h1 outputs by (index mod 156) therefore turns W1*m1 into a banded
matrix: a tile of 112 consecutive (permuted) outputs only contracts over
~215 consecutive (permuted) features.  The outputs are split into 7
tiles of 112 (padded to 128 lanes) and the features are laid into 8
SBUF "slots" of 128 rows by a greedy chain such that tile t's window is
covered by slots (t, t+1).  Layer 1 is then 7 single DoubleRow matmuls
(K = 256) per 512-column batch block instead of a dense 784-row
contraction — 2.6x fewer PE instructions.

Layer 2 contracts all 7 h1 slots (3 DoubleRow + 1 plain matmul).  It is
emitted one block late so its matmuls never head the in-order PE queue
waiting on h1 evacuations.  Layer 3 + log_softmax run in bf16/fp32 per
4-block group.  x is stored block-major in DRAM so each group's DMA
moves >=4KB-contiguous runs per partition.

With the matmul count cut ~2.6x, the kernel is bound by PSUM
evacuation: only the Vector and Scalar engines can read PSUM, at
~1.0-1.3 ns/element plus ~90ns (DVE) / ~350ns (ACT) per instruction.
Hence: L1 PSUM is grouped (2+2+2+1 banks) so seven tile evacuations
become four wide ops (biases are zero per the input spec; a general
per-tile path is kept as fallback); the softmax ln and final subtract
are deferred to a single epilogue so the scalar activation table loads
only twice (Exp / Ln) instead of thrashing 1.3us per group; and the
matmuls are emitted in the evacuation-completion order of the previous
block's banks.

The masked weights are pre-scaled by 32 (h1 stored at scale 32, h2 at
1024); the scales fold into the relu / softmax stages.  End-to-end max
relative error vs the fp32 reference is ~3e-4 (identical to the dense
version).
"""

import numpy as np
import ml_dtypes

import concourse.bass as bass
import concourse.mybir as mybir
import concourse.tile as tile
from concourse import bacc
from concourse.bass_utils import run_bass_kernel_spmd

BF16 = ml_dtypes.bfloat16
FP8 = ml_dtypes.float8_e4m3
F32 = np.float32

N_CORES = 8
B = 65536
S = B // N_CORES          # batch rows per core
IN_F = 784
PERIOD = 156              # stripe period of the 784x784 butterfly mask
NT = 7                    # h1 output tiles (112 real outputs each)
TR = 112                  # real outputs per tile
NS = 8                    # x feature slots (chain: tile t reads slots t, t+1)
H2 = 128
NCLS = 10
NSMX = 16                 # layer-3 batch tiles per softmax group
NGRP = 4                  # softmax groups per core
BLKC = S // NGRP          # batch columns per group (2048)
NBLK = S // 512           # 512-column batch blocks per core (16)

SW = 32.0                 # fp8 weight pre-scale; h1 at scale SW, h2 at SW*SW

WINDOW, STRIPES, STEP = 10, 5, 3

_CACHE = {}


def _butterfly_mask(out_f, in_f, window=WINDOW, stripes=STRIPES, step=STEP):
    i = np.arange(out_f)[:, None]
    j = np.arange(in_f)[None, :]
    jc = (i * in_f) // out_f
    band = np.abs(j - jc) <= window
    period = max(in_f // stripes, 1)
    stripe = ((j - jc) % period) < step
    return (band | stripe).astype(np.float32)


def _build_layout():
    """o_tiles: 7 lists of 112 output ids (residue-sorted); slots: 8 lists
    of 128 feature ids (-1 = pad) covering tile t's window in slots t,t+1."""
    o = np.arange(IN_F)
    out_perm = o[np.lexsort((o // PERIOD, o % PERIOD))]
    o_tiles = [out_perm[TR * t: TR * (t + 1)] for t in range(NT)]

    wins = []
    for t in range(NT):
        r = np.sort(np.unique(o_tiles[t] % PERIOD))
        wc = np.array([(c % PERIOD) for c in range(r[0] - 10, r[-1] + 11)])
        Wt = np.arange(IN_F)[np.isin(np.arange(IN_F) % PERIOD, wc)]
        wins.append(Wt[np.lexsort((Wt // PERIOD, Wt % PERIOD))])

    slots = [None] * NS
    w1set = set(wins[1].tolist())
    first = [j for j in wins[0] if j not in w1set]
    slots[0] = np.array(first + [-1] * (128 - len(first)))
    for t in range(NT):
        in_prev = set(slots[t][slots[t] >= 0].tolist())
        rest = [j for j in wins[t] if j not in in_prev]
        assert len(rest) <= 128
        slots[t + 1] = np.array(rest + [-1] * (128 - len(rest)))
    return o_tiles, slots


def _build_nc(ZB):
    nc = bacc.Bacc("TRN2", target_bir_lowering=False, debug=False, num_devices=N_CORES)

    # x block-major: [part, block, slot, col] so a group DMA moves 16KB
    # contiguous per partition.  Weights/bias host-packed per the layout.
    xq = nc.dram_tensor("xq", [128, NBLK, NS, 512], mybir.dt.float8e4, kind="ExternalInput")
    w1q = nc.dram_tensor("w1q", [128, NT * 2 * 128], mybir.dt.float8e4, kind="ExternalInput")
    w2q = nc.dram_tensor("w2q", [128, NT * H2], mybir.dt.float8e4, kind="ExternalInput")
    w3q = nc.dram_tensor("w3q", [H2, NCLS], mybir.dt.bfloat16, kind="ExternalInput")
    bias = nc.dram_tensor("bias", [128, NT + 1 + NCLS], mybir.dt.float32, kind="ExternalInput")
    out = nc.dram_tensor("out", [S, NCLS], mybir.dt.float32, kind="ExternalOutput")

    Relu = mybir.ActivationFunctionType.Relu
    Copy = mybir.ActivationFunctionType.Copy
    Exp = mybir.ActivationFunctionType.Exp
    Ln = mybir.ActivationFunctionType.Ln
    X = mybir.AxisListType.X
    DR = mybir.MatmulPerfMode.DoubleRow
    ADD = mybir.AluOpType.add
    MAX = mybir.AluOpType.max
    MULT = mybir.AluOpType.mult

    with tile.TileContext(nc) as tc:
        with (
            tc.tile_pool(name="consts", bufs=1) as consts,
            tc.tile_pool(name="spool", bufs=3) as spool,
            tc.tile_pool(name="psum", bufs=1, space="PSUM") as psum,
        ):
            # PE warm-up: dummy matmuls during the initial DMA wait flip the
            # HAM clock gate to full rate before the real matmuls arrive.
            warm = consts.tile([128, 512], mybir.dt.float8e4)
            nc.vector.memset(warm[:], 0.0)
            warm_ps = psum.tile([128, 512], mybir.dt.float32, tag="l2", bufs=2)
            for i in range(4):
                nc.tensor.matmul(
                    warm_ps[:],
                    warm[:, 0:128],
                    warm[:],
                    start=(i == 0),
                    stop=(i == 3),
                    skip_group_check=True,
                )

            # weights/x interleaved so the first output tiles' inputs land
            # quickly; remaining x streams in behind compute.
            w1r = w1q.rearrange("p (t s o) -> p t s o", t=NT, s=2)
            w1_sb = consts.tile([128, NT, 2, 128], mybir.dt.float8e4)
            xt_all = consts.tile([128, NBLK, NS, 512], mybir.dt.float8e4)
            # first-block inputs issued from four engines in parallel so the
            # transfers all start right after queue bring-up (~7us)
            nc.sync.dma_start(xt_all[:, 0, 0:3], xq[:, 0, 0:3])
            nc.scalar.dma_start(w1_sb[:, 0:2], w1r[:, 0:2])
            nc.gpsimd.dma_start(w1_sb[:, 2:7], w1r[:, 2:7])
            nc.sync.dma_start(xt_all[:, 0, 3:8], xq[:, 0, 3:8])
            nc.scalar.dma_start(xt_all[:, 1], xq[:, 1])

            w2_sb = consts.tile([128, NT, H2], mybir.dt.float8e4)
            nc.sync.dma_start(w2_sb[:], w2q.rearrange("p (k o) -> p k o", k=NT))
            w3_sb = consts.tile([128, NCLS], mybir.dt.bfloat16)
            nc.sync.dma_start(w3_sb[:], w3q[:, :])
            bias_sb = consts.tile([128, NT + 1 + NCLS], mybir.dt.float32)
            nc.sync.dma_start(bias_sb[:], bias[:, :])
            b1_sb = bias_sb[:, 0:NT]
            b2_sb = bias_sb[:, NT : NT + 1]
            b3_sb = bias_sb[:, NT + 1 :]

            nc.sync.dma_start(xt_all[:, 2], xq[:, 2])
            nc.sync.dma_start(xt_all[:, 3], xq[:, 3])
            for g in range(1, NGRP):
                nc.sync.dma_start(xt_all[:, 4 * g : 4 * g + 4], xq[:, 4 * g : 4 * g + 4])

            # persistent whole-shard activations + deferred-softmax state
            h1_all = consts.tile([128, NT, S], mybir.dt.float8e4)
            h2_all = consts.tile([128, S], mybir.dt.bfloat16)
            z_all = consts.tile([128, NGRP, NSMX, NCLS], mybir.dt.float32)
            se_all = consts.tile([128, NGRP, NSMX], mybir.dt.float32)

            def l2_evac(ps_prev, ns_prev, parity):
                # psum = SW^2 * (h1 @ W2m.T); h2 stored at scale SW^2.
                # Always on scalar: vector carries the two double-bank L1
                # evacuations plus the L3 small ops.
                nc.scalar.activation(
                    h2_all[:, ns_prev], ps_prev[:], Relu,
                    bias=b2_sb[:, 0:1], scale=1.0,
                )

            def do_l3(nb_p):
                # ---- layer 3 (bf16): logits, z, exp, rowsum.  ln and the
                # final subtraction are deferred to the epilogue so the
                # scalar activation table loads only twice (Exp body / Ln
                # end) instead of thrashing Exp<->Ln every group (1.3us per
                # load).  Logits are O(1): exp needs no max-subtraction.
                # Groups 0-2 are processed 16 batch-tiles at once when their
                # last block's h2 lands (amortizes the ~350ns scalar and
                # ~90ns vector per-op overheads); the last group goes
                # block-by-block so the end-of-kernel dependency chain is a
                # quarter as long.
                if nb_p < 4 * (NGRP - 1):
                    if nb_p % 4 != 3:
                        return
                    g, bts = nb_p // 4, range(NSMX)
                else:
                    g, bts = NGRP - 1, range((nb_p % 4) * 4, (nb_p % 4) * 4 + 4)
                nbt = len(bts)
                ps_l = psum.tile([128, nbt, NCLS], mybir.dt.float32, tag="d01", bufs=1)
                for i, bt in enumerate(bts):
                    bt_abs = g * NSMX + bt
                    nc.tensor.matmul(
                        ps_l[:, i, :],
                        h2_all[:, bt_abs * 128 : (bt_abs + 1) * 128],
                        w3_sb[:, :],
                        start=(i == 0),
                        stop=(i == nbt - 1),
                        skip_group_check=True,
                    )
                zs = z_all[:, g, bts[0] : bts[0] + nbt]
                nc.vector.scalar_tensor_tensor(
                    zs, ps_l[:], 1.0 / (SW * SW),
                    b3_sb[:, None, :].to_broadcast((128, nbt, NCLS)),
                    MULT, ADD,
                )
                e = spool.tile([128, nbt, NCLS], mybir.dt.float32, tag="e")
                nc.scalar.activation(e[:], zs, Exp)
                nc.vector.reduce_sum(se_all[:, g, bts[0] : bts[0] + nbt], e[:], axis=X)

            def epilogue():
                # ln of all rowsums, one wide subtraction, one output DMA.
                # Keeping this a single chain of three instructions (plus
                # one unavoidable Ln table load) minimizes the end-of-kernel
                # serial tail.
                lse = spool.tile([128, NGRP, NSMX], mybir.dt.float32, tag="lse")
                nc.scalar.activation(lse[:], se_all[:], Ln)
                og = spool.tile([128, NGRP, NSMX, NCLS], mybir.dt.float32, tag="og")
                nc.vector.tensor_sub(
                    og[:],
                    z_all[:],
                    lse[:, :, :, None].to_broadcast((128, NGRP, NSMX, NCLS)),
                )
                # batch is host-permuted so partition p owns 64 globally
                # consecutive output rows -> one 2560B contiguous run per
                # partition
                nc.sync.dma_start(
                    out[:, :].rearrange("(p g bt) c -> p g bt c", p=128, g=NGRP),
                    og[:],
                )

            def l2_mms(nb_p, ns_p):
                ps_l2 = psum.tile([128, 512], mybir.dt.float32, tag="l2", bufs=2)
                for p in range(3):
                    nc.tensor.matmul(
                        ps_l2[:],
                        w2_sb[:, 2 * p : 2 * p + 2, :],
                        h1_all[:, 2 * p : 2 * p + 2, ns_p],
                        start=(p == 0),
                        stop=False,
                        perf_mode=DR,
                    )
                nc.tensor.matmul(
                    ps_l2[:],
                    w2_sb[:, 6, :],
                    h1_all[:, 6, ns_p],
                    start=False,
                    stop=True,
                )
                return ps_l2

            # Layer 2 for block nb is emitted one iteration later (during
            # nb+1's layer 1) so its matmuls never sit at the head of the
            # in-order PE queue waiting for h1 evacuations.
            pending = None  # (ns, nb) whose layer 2 is not yet emitted
            for nb in range(NBLK):
                ns = slice(nb * 512, (nb + 1) * 512)

                # ---- layer 1: 7 banded DoubleRow matmuls, fp8.  PSUM is
                # grouped 2+2+2+1 banks so the evacuations below can be 4
                # wide ops instead of 7 (psum-read rate is the kernel
                # bottleneck; op overhead is ~90ns DVE / ~350ns ACT).  The
                # t6/l2 tag double-buffers, giving the only rotation slack
                # the 8-bank budget allows.  Matmuls are emitted in evac-
                # completion order of the previous block's banks (d01 first
                # on vector, then d45 on scalar, then d23) so the in-order
                # PE queue stalls as little as possible.
                d01 = psum.tile([128, 2, 512], mybir.dt.float32, tag="d01")
                d23 = psum.tile([128, 2, 512], mybir.dt.float32, tag="d23")
                d45 = psum.tile([128, 2, 512], mybir.dt.float32, tag="d45")
                t6 = psum.tile([128, 512], mybir.dt.float32, tag="l2", bufs=2)
                slices = [d01[:, 0], d01[:, 1], d23[:, 0], d23[:, 1],
                          d45[:, 0], d45[:, 1], t6[:]]
                for t in [0, 1, 4, 5, 6, 2, 3]:
                    nc.tensor.matmul(
                        slices[t],
                        w1_sb[:, t, :, :],
                        xt_all[:, nb, t : t + 2, :],
                        start=True,
                        stop=True,
                        perf_mode=DR,
                        skip_group_check=True,
                    )
                # delayed layer-2 matmuls for the previous block
                ps_l2 = None
                if pending is not None:
                    ns_p, nb_p = pending
                    ps_l2 = l2_mms(nb_p, ns_p)
                # psum = SW * (x @ W1m.T); h1 stored = relu(psum + SW*b1)
                # = SW * relu(true + b1).  With zero biases the evacuations
                # merge into two double-bank ops (vector) and one triple
                # (scalar); otherwise per-tile with the per-partition bias.
                if ZB:
                    nc.vector.tensor_scalar(
                        h1_all[:, 0:2, ns], d01[:], 0.0, 0.0, ADD, MAX
                    )
                    nc.scalar.activation(
                        h1_all[:, 4:6, ns], d45[:], Relu, bias=0.0, scale=1.0
                    )
                    nc.scalar.activation(
                        h1_all[:, 6, ns], t6[:], Relu, bias=0.0, scale=1.0
                    )
                    nc.vector.tensor_scalar(
                        h1_all[:, 2:4, ns], d23[:], 0.0, 0.0, ADD, MAX
                    )
                else:
                    for t in range(NT):
                        h1_dst = h1_all[:, t, ns]
                        if (t + nb) % 2 == 0:
                            nc.vector.tensor_scalar(
                                h1_dst, slices[t], b1_sb[:, t : t + 1], 0.0, ADD, MAX
                            )
                        else:
                            nc.scalar.activation(
                                h1_dst, slices[t], Relu,
                                bias=b1_sb[:, t : t + 1], scale=1.0,
                            )
                if ps_l2 is not None:
                    l2_evac(ps_l2, ns_p)
                    if nb_p % 4 == 3:
                        do_l3(nb_p // 4)
                pending = (ns, nb)

            # flush: final block's layer 2 + last softmax group + epilogue
            ns_p, nb_p = pending
            ps_l2 = l2_mms(nb_p, ns_p)
            l2_evac(ps_l2, ns_p)
            do_l3(NGRP - 1)
            epilogue()

    return nc


def _shard_perm():
    """Shard position g*2048 + bt*128 + p processes original row
    p*64 + g*16 + bt, so each partition owns 64 consecutive output rows
    (one contiguous 2560B DMA run per partition)."""
    pos = np.arange(S)
    g, r = np.divmod(pos, NSMX * 128)
    bt, p = np.divmod(r, 128)
    return p * (NGRP * NSMX) + g * NSMX + bt


def _prep_inputs(x, W1, b1, W2, b2, W3, b3):
    m1 = _butterfly_mask(IN_F, IN_F)
    m2 = _butterfly_mask(H2, IN_F)
    m3 = _butterfly_mask(NCLS, H2)
    o_tiles, slots = _build_layout()

    w1t = (np.asarray(W1, F32) * m1).T * SW     # [j, o]
    w2t = (np.asarray(W2, F32) * m2).T * SW     # [j, o2]

    # w1 layout [p, t, s, o]: weight of feature slots[t+s][p] for output
    # o_tiles[t][o]; zero at pads.
    w1l = np.zeros((128, NT, 2, 128), dtype=F32)
    for t in range(NT):
        cols = o_tiles[t]
        for s in range(2):
            rows = slots[t + s]
            valid = rows >= 0
            w1l[valid, t, s, :TR] = w1t[np.ix_(rows[valid], cols)]
    w1l = np.ascontiguousarray(w1l.reshape(128, NT * 2 * 128)).astype(FP8)

    # w2 layout [p, k, o2]: weight of h1 feature o_tiles[k][p]
    w2l = np.zeros((128, NT, H2), dtype=F32)
    for k in range(NT):
        w2l[:TR, k, :] = w2t[o_tiles[k], :]
    w2l = np.ascontiguousarray(w2l.reshape(128, NT * H2)).astype(FP8)

    w3l = ((np.asarray(W3, F32) * m3).T).astype(BF16).copy()

    # bias pack [128, 7 + 1 + 10] f32: b1 per tile (scaled by SW), b2
    # scaled by SW^2, b3 broadcast.
    biasp = np.zeros((128, NT + 1 + NCLS), F32)
    b1f = np.asarray(b1, F32) * SW
    for t in range(NT):
        biasp[:TR, t] = b1f[o_tiles[t]]
    biasp[:, NT] = np.asarray(b2, F32) * (SW * SW)
    biasp[:, NT + 1 :] = np.asarray(b3, F32)[None, :]
    biasp = np.ascontiguousarray(biasp)

    # x: [B, 784] -> fp8 slot layout, batch permuted within each group,
    # block-major per core: xq[p, blk, slot, col]
    perm = _shard_perm()
    full_perm = np.concatenate([c * S + perm for c in range(N_CORES)])
    xT = np.asarray(x, F32).T.astype(FP8)[:, full_perm]    # [784, B]
    xs = np.zeros((NS, 128, B), dtype=FP8)
    for s in range(NS):
        rows = slots[s]
        valid = rows >= 0
        xs[s, valid] = xT[rows[valid]]

    in_maps = []
    for c in range(N_CORES):
        xc = xs[:, :, c * S : (c + 1) * S].reshape(NS, 128, NBLK, 512)
        xc = np.ascontiguousarray(xc.transpose(1, 2, 0, 3))   # [p, blk, s, col]
        in_maps.append(
            {
                "xq": xc,
                "w1q": w1l,
                "w2q": w2l,
                "w3q": w3l,
                "bias": biasp,
            }
        )
    return in_maps


def _run(inputs, trace=False, **run_kwargs):
    zb = bool(
        np.all(np.asarray(inputs["b1"]) == 0) and np.all(np.asarray(inputs["b2"]) == 0)
    )
    key = f"nc{zb}"
    if key not in _CACHE:
        nc = _build_nc(zb)
        nc.finalize()
        _CACHE[key] = nc
    nc = _CACHE[key]
    in_maps = _prep_inputs(**inputs)
    res = run_bass_kernel_spmd(
        nc,
        in_maps,
        core_ids=list(range(N_CORES)),
        trace=trace,
        **run_kwargs,
    )
    out = np.concatenate([r["out"] for r in res.results], axis=0)
    return out, res


def kernel(**inputs):
    out, _ = _run(inputs, trace=False)
    return out


# revision 31
# speedup vs baseline: 1.0993x; 1.0993x over previous
"""ButterflyMlp Trainium2 kernel (banded-permutation version).

Reference computation (B=65536):
    h1 = relu(x @ (W1*m1).T + b1)          # [B, 784]
    h2 = relu(h1 @ (W2*m2).T + b2)         # [B, 128]
    logits = h2 @ (W3*m3).T + b3           # [B, 10]
    out = log_softmax(logits, axis=1)

Strategy: pure data parallel over 8 NeuronCores (batch sharded 8192/core,
masked weights replicated), fp8e4m3 with fp32 PSUM accumulation.

Every nonzero offset d = j - o of the butterfly mask m1 satisfies
(d mod 156) in [-10, 10]: the band |d| <= 10 trivially, and the stripes
have d mod 156 in {0,1,2}.  Sorting both the 784 input features and the
784 h1 outputs by (index mod 156) therefore turns W1*m1 into a banded
matrix: a tile of 112 consecutive (permuted) outputs only contracts over
~215 consecutive (permuted) features.  The outputs are split into 7
tiles of 112 (padded to 128 lanes) and the features are laid into 8
SBUF "slots" of 128 rows by a greedy chain such that tile t's window is
covered by slots (t, t+1).  Layer 1 is then 7 single DoubleRow matmuls
(K = 256) per 512-column batch block instead of a dense 784-row
contraction — 2.6x fewer PE instructions.

Layer 2 contracts all 7 h1 slots (3 DoubleRow + 1 plain matmul).  It is
emitted one block late so its matmuls never head the in-order PE queue
waiting on h1 evacuations.  Layer 3 + log_softmax run in bf16/fp32 per
4-block group.  x is stored block-major in DRAM so each group's DMA
moves >=4KB-contiguous runs per partition.

With the matmul count cut ~2.6x, the kernel is bound by PSUM
evacuation: only the Vector and Scalar engines can read PSUM, at
~1.0-1.3 ns/element plus ~90ns (DVE) / ~350ns (ACT) per instruction.
Hence: L1 PSUM is grouped (2+2+2+1 banks) so seven tile evacuations
become four wide ops (biases are zero per the input spec; a general
per-tile path is kept as fallback); the softmax ln and final subtract
are deferred to a single epilogue so the scalar activation table loads
only twice (Exp / Ln) instead of thrashing 1.3us per group; and the
matmuls are emitted in the evacuation-completion order of the previous
block's banks.

The masked weights are pre-scaled by 32 (h1 stored at scale 32, h2 at
1024); the scales fold into the relu / softmax stages.  End-to-end max
relative error vs the fp32 reference is ~3e-4 (identical to the dense
version).
"""

import numpy as np
import ml_dtypes

import concourse.bass as bass
import concourse.mybir as mybir
import concourse.tile as tile
from concourse import bacc
from concourse.bass_utils import run_bass_kernel_spmd

BF16 = ml_dtypes.bfloat16
FP8 = ml_dtypes.float8_e4m3
F32 = np.float32

N_CORES = 8
B = 65536
S = B // N_CORES          # batch rows per core
IN_F = 784
PERIOD = 156              # stripe period of the 784x784 butterfly mask
NT = 7                    # h1 output tiles (112 real outputs each)
TR = 112                  # real outputs per tile
NS = 8                    # x feature slots (chain: tile t reads slots t, t+1)
H2 = 128
NCLS = 10
NSMX = 16                 # layer-3 batch tiles per softmax group
NGRP = 4                  # softmax groups per core
BLKC = S // NGRP          # batch columns per group (2048)
NBLK = S // 512           # 512-column batch blocks per core (16)

SW = 32.0                 # fp8 weight pre-scale; h1 at scale SW, h2 at SW*SW

WINDOW, STRIPES, STEP = 10, 5, 3

_CACHE = {}


def _butterfly_mask(out_f, in_f, window=WINDOW, stripes=STRIPES, step=STEP):
    i = np.arange(out_f)[:, None]
    j = np.arange(in_f)[None, :]
    jc = (i * in_f) // out_f
    band = np.abs(j - jc) <= window
    period = max(in_f // stripes, 1)
    stripe = ((j - jc) % period) < step
    return (band | stripe).astype(np.float32)


def _build_layout():
    """o_tiles: 7 lists of 112 output ids (residue-sorted); slots: 8 lists
    of 128 feature ids (-1 = pad) covering tile t's window in slots t,t+1."""
    o = np.arange(IN_F)
    out_perm = o[np.lexsort((o // PERIOD, o % PERIOD))]
    o_tiles = [out_perm[TR * t: TR * (t + 1)] for t in range(NT)]

    wins = []
    for t in range(NT):
        r = np.sort(np.unique(o_tiles[t] % PERIOD))
        wc = np.array([(c % PERIOD) for c in range(r[0] - 10, r[-1] + 11)])
        Wt = np.arange(IN_F)[np.isin(np.arange(IN_F) % PERIOD, wc)]
        wins.append(Wt[np.lexsort((Wt // PERIOD, Wt % PERIOD))])

    slots = [None] * NS
    w1set = set(wins[1].tolist())
    first = [j for j in wins[0] if j not in w1set]
    slots[0] = np.array(first + [-1] * (128 - len(first)))
    for t in range(NT):
        in_prev = set(slots[t][slots[t] >= 0].tolist())
        rest = [j for j in wins[t] if j not in in_prev]
        assert len(rest) <= 128
        slots[t + 1] = np.array(rest + [-1] * (128 - len(rest)))
    return o_tiles, slots


def _build_nc(ZB):
    nc = bacc.Bacc("TRN2", target_bir_lowering=False, debug=False, num_devices=N_CORES)

    # x block-major: [part, block, slot, col] so a group DMA moves 16KB
    # contiguous per partition.  Weights/bias host-packed per the layout.
    xq = nc.dram_tensor("xq", [128, NBLK, NS, 512], mybir.dt.float8e4, kind="ExternalInput")
    w1q = nc.dram_tensor("w1q", [128, NT * 2 * 128], mybir.dt.float8e4, kind="ExternalInput")
    w2q = nc.dram_tensor("w2q", [128, NT * H2], mybir.dt.float8e4, kind="ExternalInput")
    w3q = nc.dram_tensor("w3q", [H2, NCLS], mybir.dt.bfloat16, kind="ExternalInput")
    bias = nc.dram_tensor("bias", [128, NT + 1 + NCLS], mybir.dt.float32, kind="ExternalInput")
    out = nc.dram_tensor("out", [S, NCLS], mybir.dt.float32, kind="ExternalOutput")

    Relu = mybir.ActivationFunctionType.Relu
    Copy = mybir.ActivationFunctionType.Copy
    Exp = mybir.ActivationFunctionType.Exp
    Ln = mybir.ActivationFunctionType.Ln
    X = mybir.AxisListType.X
    DR = mybir.MatmulPerfMode.DoubleRow
    ADD = mybir.AluOpType.add
    MAX = mybir.AluOpType.max
    MULT = mybir.AluOpType.mult

    with tile.TileContext(nc) as tc:
        with (
            tc.tile_pool(name="consts", bufs=1) as consts,
            tc.tile_pool(name="spool", bufs=3) as spool,
            tc.tile_pool(name="psum", bufs=1, space="PSUM") as psum,
        ):
            # PE warm-up: dummy matmuls during the initial DMA wait flip the
            # HAM clock gate to full rate before the real matmuls arrive.
            warm = consts.tile([128, 512], mybir.dt.float8e4)
            nc.vector.memset(warm[:], 0.0)
            warm_ps = psum.tile([128, 512], mybir.dt.float32, tag="l2", bufs=2)
            for i in range(4):
                nc.tensor.matmul(
                    warm_ps[:],
                    warm[:, 0:128],
                    warm[:],
                    start=(i == 0),
                    stop=(i == 3),
                    skip_group_check=True,
                )

            # weights/x interleaved so the first output tiles' inputs land
            # quickly; remaining x streams in behind compute.
            w1r = w1q.rearrange("p (t s o) -> p t s o", t=NT, s=2)
            w1_sb = consts.tile([128, NT, 2, 128], mybir.dt.float8e4)
            xt_all = consts.tile([128, NBLK, NS, 512], mybir.dt.float8e4)
            # first-block inputs issued from four engines in parallel so the
            # transfers all start right after queue bring-up (~7us)
            nc.sync.dma_start(xt_all[:, 0, 0:3], xq[:, 0, 0:3])
            nc.scalar.dma_start(w1_sb[:, 0:2], w1r[:, 0:2])
            nc.gpsimd.dma_start(w1_sb[:, 2:7], w1r[:, 2:7])
            nc.sync.dma_start(xt_all[:, 0, 3:8], xq[:, 0, 3:8])
            nc.scalar.dma_start(xt_all[:, 1], xq[:, 1])

            w2_sb = consts.tile([128, NT, H2], mybir.dt.float8e4)
            nc.sync.dma_start(w2_sb[:], w2q.rearrange("p (k o) -> p k o", k=NT))
            w3_sb = consts.tile([128, NCLS], mybir.dt.bfloat16)
            nc.sync.dma_start(w3_sb[:], w3q[:, :])
            bias_sb = consts.tile([128, NT + 1 + NCLS], mybir.dt.float32)
            nc.sync.dma_start(bias_sb[:], bias[:, :])
            b1_sb = bias_sb[:, 0:NT]
            b2_sb = bias_sb[:, NT : NT + 1]
            b3_sb = bias_sb[:, NT + 1 :]

            nc.sync.dma_start(xt_all[:, 2], xq[:, 2])
            nc.sync.dma_start(xt_all[:, 3], xq[:, 3])
            for g in range(1, NGRP):
                nc.sync.dma_start(xt_all[:, 4 * g : 4 * g + 4], xq[:, 4 * g : 4 * g + 4])

            # persistent whole-shard activations + deferred-softmax state
            h1_all = consts.tile([128, NT, S], mybir.dt.float8e4)
            h2_all = consts.tile([128, S], mybir.dt.bfloat16)
            z_all = consts.tile([128, NGRP, NSMX, NCLS], mybir.dt.float32)
            se_all = consts.tile([128, NGRP, NSMX], mybir.dt.float32)

            def l2_evac(ps_prev, ns_prev, parity):
                # psum = SW^2 * (h1 @ W2m.T); h2 stored at scale SW^2.
                # Always on scalar: vector carries the two double-bank L1
                # evacuations plus the L3 small ops.
                nc.scalar.activation(
                    h2_all[:, ns_prev], ps_prev[:], Relu,
                    bias=b2_sb[:, 0:1], scale=1.0,
                )

            def do_l3(nb_p):
                # ---- layer 3 (bf16): logits, z, exp, rowsum.  ln and the
                # final subtraction are deferred to the epilogue so the
                # scalar activation table loads only twice (Exp body / Ln
                # end) instead of thrashing Exp<->Ln every group (1.3us per
                # load).  Logits are O(1): exp needs no max-subtraction.
                # Groups 0-2 are processed 16 batch-tiles at once when their
                # last block's h2 lands (amortizes the ~350ns scalar and
                # ~90ns vector per-op overheads); the last group goes
                # block-by-block so the end-of-kernel dependency chain is a
                # quarter as long.
                if nb_p < 4 * (NGRP - 1):
                    if nb_p % 4 != 3:
                        return
                    g, bts = nb_p // 4, range(NSMX)
                else:
                    g, bts = NGRP - 1, range((nb_p % 4) * 4, (nb_p % 4) * 4 + 4)
                nbt = len(bts)
                ps_l = psum.tile([128, nbt, NCLS], mybir.dt.float32, tag="l2", bufs=2)
                for i, bt in enumerate(bts):
                    bt_abs = g * NSMX + bt
                    nc.tensor.matmul(
                        ps_l[:, i, :],
                        h2_all[:, bt_abs * 128 : (bt_abs + 1) * 128],
                        w3_sb[:, :],
                        start=(i == 0),
                        stop=(i == nbt - 1),
                        skip_group_check=True,
                    )
                zs = z_all[:, g, bts[0] : bts[0] + nbt]
                nc.vector.scalar_tensor_tensor(
                    zs, ps_l[:], 1.0 / (SW * SW),
                    b3_sb[:, None, :].to_broadcast((128, nbt, NCLS)),
                    MULT, ADD,
                )
                e = spool.tile([128, nbt, NCLS], mybir.dt.float32, tag="e")
                nc.scalar.activation(e[:], zs, Exp)
                nc.vector.reduce_sum(se_all[:, g, bts[0] : bts[0] + nbt], e[:], axis=X)

            def epilogue():
                # ln of all rowsums, one wide subtraction, one output DMA.
                # Keeping this a single chain of three instructions (plus
                # one unavoidable Ln table load) minimizes the end-of-kernel
                # serial tail.
                lse = spool.tile([128, NGRP, NSMX], mybir.dt.float32, tag="lse")
                nc.scalar.activation(lse[:], se_all[:], Ln)
                og = spool.tile([128, NGRP, NSMX, NCLS], mybir.dt.float32, tag="og")
                nc.vector.tensor_sub(
                    og[:],
                    z_all[:],
                    lse[:, :, :, None].to_broadcast((128, NGRP, NSMX, NCLS)),
                )
                # batch is host-permuted so partition p owns 64 globally
                # consecutive output rows -> one 2560B contiguous run per
                # partition
                nc.sync.dma_start(
                    out[:, :].rearrange("(p g bt) c -> p g bt c", p=128, g=NGRP),
                    og[:],
                )

            def l2_mms(nb_p, ns_p):
                ps_l2 = psum.tile([128, 512], mybir.dt.float32, tag="l2", bufs=2)
                for p in range(3):
                    nc.tensor.matmul(
                        ps_l2[:],
                        w2_sb[:, 2 * p : 2 * p + 2, :],
                        h1_all[:, 2 * p : 2 * p + 2, ns_p],
                        start=(p == 0),
                        stop=False,
                        perf_mode=DR,
                    )
                nc.tensor.matmul(
                    ps_l2[:],
                    w2_sb[:, 6, :],
                    h1_all[:, 6, ns_p],
                    start=False,
                    stop=True,
                )
                return ps_l2

            # Layer 2 for block nb is emitted one iteration later (during
            # nb+1's layer 1) so its matmuls never sit at the head of the
            # in-order PE queue waiting for h1 evacuations.
            pending = None  # (ns, nb) whose layer 2 is not yet emitted
            for nb in range(NBLK):
                ns = slice(nb * 512, (nb + 1) * 512)

                # ---- layer 1: 7 banded DoubleRow matmuls, fp8.  PSUM is
                # grouped 2+2+2+1 banks so the evacuations below can be 4
                # wide ops instead of 7 (psum-read rate is the kernel
                # bottleneck; op overhead is ~90ns DVE / ~350ns ACT).  The
                # t6/l2 tag double-buffers, giving the only rotation slack
                # the 8-bank budget allows.  Matmuls are emitted in evac-
                # completion order of the previous block's banks (d01 first
                # on vector, then d45 on scalar, then d23) so the in-order
                # PE queue stalls as little as possible.
                d01 = psum.tile([128, 2, 512], mybir.dt.float32, tag="d01")
                d23 = psum.tile([128, 2, 512], mybir.dt.float32, tag="d23")
                d45 = psum.tile([128, 2, 512], mybir.dt.float32, tag="d45")
                t6 = psum.tile([128, 512], mybir.dt.float32, tag="l2", bufs=2)
                slices = [d01[:, 0], d01[:, 1], d23[:, 0], d23[:, 1],
                          d45[:, 0], d45[:, 1], t6[:]]
                for t in [0, 1, 4, 5, 6, 2, 3]:
                    nc.tensor.matmul(
                        slices[t],
                        w1_sb[:, t, :, :],
                        xt_all[:, nb, t : t + 2, :],
                        start=True,
                        stop=True,
                        perf_mode=DR,
                        skip_group_check=True,
                    )
                # delayed layer-2 matmuls for the previous block
                ps_l2 = None
                if pending is not None:
                    ns_p, nb_p = pending
                    ps_l2 = l2_mms(nb_p, ns_p)
                # psum = SW * (x @ W1m.T); h1 stored = relu(psum + SW*b1)
                # = SW * relu(true + b1).  With zero biases the evacuations
                # merge into two double-bank ops (vector) and one triple
                # (scalar); otherwise per-tile with the per-partition bias.
                if ZB:
                    nc.vector.tensor_scalar(
                        h1_all[:, 0:2, ns], d01[:], 0.0, 0.0, ADD, MAX
                    )
                    nc.scalar.activation(
                        h1_all[:, 4:6, ns], d45[:], Relu, bias=0.0, scale=1.0
                    )
                    nc.scalar.activation(
                        h1_all[:, 6, ns], t6[:], Relu, bias=0.0, scale=1.0
                    )
                    nc.vector.tensor_scalar(
                        h1_all[:, 2:4, ns], d23[:], 0.0, 0.0, ADD, MAX
                    )
                else:
                    for t in range(NT):
                        h1_dst = h1_all[:, t, ns]
                        if (t + nb) % 2 == 0:
                            nc.vector.tensor_scalar(
                                h1_dst, slices[t], b1_sb[:, t : t + 1], 0.0, ADD, MAX
                            )
                        else:
                            nc.scalar.activation(
                                h1_dst, slices[t], Relu,
                                bias=b1_sb[:, t : t + 1], scale=1.0,
                            )
                if ps_l2 is not None:
                    l2_evac(ps_l2, ns_p)
                    if nb_p % 4 == 3:
                        do_l3(nb_p // 4)
                pending = (ns, nb)

            # flush: final block's layer 2 + last softmax group + epilogue
            ns_p, nb_p = pending
            ps_l2 = l2_mms(nb_p, ns_p)
            l2_evac(ps_l2, ns_p)
            do_l3(NGRP - 1)
            epilogue()

    return nc


def _shard_perm():
    """Shard position g*2048 + bt*128 + p processes original row
    p*64 + g*16 + bt, so each partition owns 64 consecutive output rows
    (one contiguous 2560B DMA run per partition)."""
    pos = np.arange(S)
    g, r = np.divmod(pos, NSMX * 128)
    bt, p = np.divmod(r, 128)
    return p * (NGRP * NSMX) + g * NSMX + bt


def _prep_inputs(x, W1, b1, W2, b2, W3, b3):
    m1 = _butterfly_mask(IN_F, IN_F)
    m2 = _butterfly_mask(H2, IN_F)
    m3 = _butterfly_mask(NCLS, H2)
    o_tiles, slots = _build_layout()

    w1t = (np.asarray(W1, F32) * m1).T * SW     # [j, o]
    w2t = (np.asarray(W2, F32) * m2).T * SW     # [j, o2]

    # w1 layout [p, t, s, o]: weight of feature slots[t+s][p] for output
    # o_tiles[t][o]; zero at pads.
    w1l = np.zeros((128, NT, 2, 128), dtype=F32)
    for t in range(NT):
        cols = o_tiles[t]
        for s in range(2):
            rows = slots[t + s]
            valid = rows >= 0
            w1l[valid, t, s, :TR] = w1t[np.ix_(rows[valid], cols)]
    w1l = np.ascontiguousarray(w1l.reshape(128, NT * 2 * 128)).astype(FP8)

    # w2 layout [p, k, o2]: weight of h1 feature o_tiles[k][p]
    w2l = np.zeros((128, NT, H2), dtype=F32)
    for k in range(NT):
        w2l[:TR, k, :] = w2t[o_tiles[k], :]
    w2l = np.ascontiguousarray(w2l.reshape(128, NT * H2)).astype(FP8)

    w3l = ((np.asarray(W3, F32) * m3).T).astype(BF16).copy()

    # bias pack [128, 7 + 1 + 10] f32: b1 per tile (scaled by SW), b2
    # scaled by SW^2, b3 broadcast.
    biasp = np.zeros((128, NT + 1 + NCLS), F32)
    b1f = np.asarray(b1, F32) * SW
    for t in range(NT):
        biasp[:TR, t] = b1f[o_tiles[t]]
    biasp[:, NT] = np.asarray(b2, F32) * (SW * SW)
    biasp[:, NT + 1 :] = np.asarray(b3, F32)[None, :]
    biasp = np.ascontiguousarray(biasp)

    # x: [B, 784] -> fp8 slot layout, batch permuted within each group,
    # block-major per core: xq[p, blk, slot, col]
    perm = _shard_perm()
    full_perm = np.concatenate([c * S + perm for c in range(N_CORES)])
    xT = np.asarray(x, F32).T.astype(FP8)[:, full_perm]    # [784, B]
    xs = np.zeros((NS, 128, B), dtype=FP8)
    for s in range(NS):
        rows = slots[s]
        valid = rows >= 0
        xs[s, valid] = xT[rows[valid]]

    in_maps = []
    for c in range(N_CORES):
        xc = xs[:, :, c * S : (c + 1) * S].reshape(NS, 128, NBLK, 512)
        xc = np.ascontiguousarray(xc.transpose(1, 2, 0, 3))   # [p, blk, s, col]
        in_maps.append(
            {
                "xq": xc,
                "w1q": w1l,
                "w2q": w2l,
                "w3q": w3l,
                "bias": biasp,
            }
        )
    return in_maps


def _run(inputs, trace=False, **run_kwargs):
    zb = bool(
        np.all(np.asarray(inputs["b1"]) == 0) and np.all(np.asarray(inputs["b2"]) == 0)
    )
    key = f"nc{zb}"
    if key not in _CACHE:
        nc = _build_nc(zb)
        nc.finalize()
        _CACHE[key] = nc
    nc = _CACHE[key]
    in_maps = _prep_inputs(**inputs)
    res = run_bass_kernel_spmd(
        nc,
        in_maps,
        core_ids=list(range(N_CORES)),
        trace=trace,
        **run_kwargs,
    )
    out = np.concatenate([r["out"] for r in res.results], axis=0)
    return out, res


def kernel(**inputs):
    out, _ = _run(inputs, trace=False)
    return out
